# revision 26
# baseline (speedup 1.0000x reference)
"""CubeAttention Trainium2 Bass kernel (8-core SPMD), v3.1.

Data-parallel over the query grid: the 20^3 grid splits into 8 slabs of
[5,10,20] (4 blocks along i x 2 halves along j). Each core processes 4
query blocks of [5i,5j,10k] (250 queries) against a 9x9x14 support
window, as 9 si-planes of [126 slots, 250 queries].

Key structure (vs the v2 291us kernel):
  - blocks of 250 queries: half the QKT/AV matmul count for the same
    streamed columns; 126-partition planes use the PE wider.
  - softmax marginals fused into the AV matmul: the s-major value table
    carries 64 static columns (ones + coordinate indicators at
    32-aligned rows), so one [126,128] x [126,250] matmul per plane
    yields AV, Z and all three axis marginals.
  - relpos-query C terms + masks computed once per core into
    block-major tables; per-block qa assembly is 4 plain 2D DMAs.
  - softmax normalisation deferred: Z rides the AV output (ones col),
    1/Z via two tiny per-block transposes, applied in the FIN drain;
    bias enters as a Z*bo contract row so nothing runs after FIN(3)
    except the output DMA.
  - no Pool-engine work (its copies/DMA-gen are SW-slow); PSUM spread
    over 4 pools (8 banks) so setup matmuls pipeline 8 drains deep.

Partition-base rule (engine APs must start at 0/32/64/96): marginal
rows sit at psum 64:96 and 96:120; the i/j fixup matmuls widen their
contract with zero lhsT rows to start at partition 0.
"""

import numpy as np

SCOPE, GN, D, CAP = 2, 20, 64, 32
NEG = np.float32(-1e9)

# wpack column layout (bf16, [65, _WCOLS])
_O_WQ, _O_WK, _O_WV, _O_WO = 0, 64, 128, 192
_O_ZI = 256            # [64, 13]  shifted-G2 (i-axis C), band cols 4:9
_O_ZJ = _O_ZI + 13     # [64, 23]  shifted-G1 (j-axis C), band cols 9:14
_O_ZK = _O_ZJ + 23     # [64, 43]  shifted-G3 (k-axis C), band cols 19:24
_O_RVI = _O_ZK + 43    # [10, 5*64]  RV2 fixup (g); row 0 zero (kills Z)
_O_RVJ = _O_RVI + 320  # [24, 10*64] RV1 fixup (5bj+g); rows 0:10 zero
_O_MI = _O_RVJ + 640   # [9, 1000]  i-axis C mask (block-major cols)
_O_MJ = _O_MI + 1000   # [14, 1000] j-axis C mask
_O_MK = _O_MJ + 1000   # [24, 1000] k-axis C mask
_O_IND = _O_MK + 1000  # [32, 3*1134] kp indicator rows (3 ping-pong)
_WCOLS = _O_IND + 3402
_W_SB = _O_IND         # columns of wpack staged in SBUF

# fconst layout (f32, [125, 1344])
_F_RVK = 64            # [24, 20*64] RV3 fixup tiles (10bk+g), rows 0:24
_FCOLS = _F_RVK + 1280

_CACHE = {}


def _bass_mod():
    if "nc" in _CACHE:
        return _CACHE["nc"]
    import sys
    for p in ("/opt/trn_rl_repo", "/root/.axon_site/_ro/trn_rl_repo"):
        if p not in sys.path:
            sys.path.append(p)
    import concourse.tile as tile
    from concourse import bacc, mybir

    f32 = mybir.dt.float32
    bf16 = mybir.dt.bfloat16
    AF = mybir.ActivationFunctionType

    nc = bacc.Bacc("TRN2", target_bir_lowering=False, debug=False)
    P = {}
    P["seT"] = nc.declare_dram_parameter("seT", [65, 3024], bf16, isOutput=False)
    P["seTk"] = nc.declare_dram_parameter("seTk", [65, 3024], bf16, isOutput=False)
    P["wpack"] = nc.declare_dram_parameter("wpack", [65, _WCOLS], bf16,
                                           isOutput=False)
    P["vind"] = nc.declare_dram_parameter("vind", [126, 24 * 64], bf16,
                                          isOutput=False)
    P["fc"] = nc.declare_dram_parameter("fc", [125, _FCOLS], f32, isOutput=False)
    out_p = nc.declare_dram_parameter("out", [8, 125, 64], f32, isOutput=True)

    with tile.TileContext(nc) as tc:
        with (
            tc.tile_pool(name="const", bufs=1) as const,
            tc.tile_pool(name="sS", bufs=8) as sS,
            tc.tile_pool(name="sVP", bufs=3) as sVP,
            tc.tile_pool(name="sQA", bufs=3) as sQA,
            tc.tile_pool(name="sW", bufs=2) as sW,
            tc.tile_pool(name="dram", bufs=1, space="DRAM") as dpool,
            tc.tile_pool(name="psS", bufs=2, space="PSUM") as psS,
            tc.tile_pool(name="psL", bufs=3, space="PSUM") as psL,
            tc.tile_pool(name="psV", bufs=2, space="PSUM") as psV,
            tc.tile_pool(name="psK", bufs=1, space="PSUM") as psK,
        ):
            # ---- input DMAs (spread across HWDGE queues, none on Pool) ----
            seT = const.tile([65, 3024], bf16, tag="seT")
            nc.sync.dma_start(seT[:], P["seT"][:])
            seT4 = seT[:].rearrange("p (i j k) -> p i j k", i=9, j=14)
            wp = const.tile([65, _W_SB], bf16, tag="wp")
            nc.scalar.dma_start(wp[:], P["wpack"][:, 0:_W_SB])
            seTk = const.tile([65, 3024], bf16, tag="seTk")
            nc.scalar.dma_start(seTk[:], P["seTk"][:])
            fc = const.tile([125, _FCOLS], f32, tag="fc")
            nc.sync.dma_start(fc[:], P["fc"][:])
            kp = const.tile([96, 3 * 1134], bf16, tag="kp")
            nc.scalar.dma_start(kp[64:96, :], P["wpack"][0:32, _O_IND:_WCOLS])
            vsm = const.tile([126, 24 * 128], bf16, tag="vsm")
            nc.sync.dma_start(
                vsm[:].rearrange("p (c e) -> p c e", c=24)[:, :, 64:128],
                P["vind"][:].rearrange("p (c e) -> p c e", c=24))
            identf = const.tile([1, 1], f32, tag="identf")
            nc.vector.memset(identf[:], 1.0)

            dcnt = [0]

            def drain(dst, src):
                # alternate PSUM->SBUF copy engines
                dcnt[0] += 1
                if dcnt[0] % 2 == 0:
                    nc.scalar.copy(dst, src)
                else:
                    nc.vector.tensor_copy(dst, src)

            bigp = [0]

            def big_tile(shape):
                bigp[0] += 1
                pool = psS if bigp[0] % 2 == 0 else psL
                tag = "x" if pool is psS else "L"
                return pool.tile(shape, f32, tag=tag, name=f"bt{bigp[0]}")

            smlp = [0]

            def small_tile(shape):
                smlp[0] += 1
                pool = psV if smlp[0] % 3 else psK
                tag = "v" if pool is psV else "k"
                return pool.tile(shape, f32, tag=tag, name=f"st{smlp[0]}")

            # reserve slot sizes: first tile request fixes nothing, but
            # keep every request within [*,504] f32 (psS/psL) / [*,250] f32
            # (psV/psK).

            # ---- Q projection, block-major (blk, a, b, c) ----
            Qb = const.tile([64, 1000], bf16, tag="Qb")
            Qb5 = Qb[:].rearrange("p (w a b c) -> p w a b c", w=4, a=5, b=5)
            for a in range(5):
                ps = small_tile([64, 200])
                nc.tensor.matmul(ps[:], wp[0:65, _O_WQ:_O_WQ + 64],
                                 seT4[:, a + 2, 2:12, 2:22],
                                 start=True, stop=True)
                for bj in range(2):
                    drain(Qb5[:, 2 * bj:2 * bj + 2, a, :, :],
                          ps[:].rearrange("p (w b k c) -> p w k b c",
                                          w=2, b=5, k=2)[:, bj, :, :, :])

            # ---- relpos-query C tables + masks (block-major) ----
            CI = const.tile([9, 1000], bf16, tag="CI")
            CI5 = CI[:].rearrange("p (w a b c) -> p w a b c", w=4, a=5, b=5)
            mi5 = wp[0:9, _O_MI:_O_MI + 1000].rearrange(
                "p (w a b c) -> p w a b c", w=4, a=5, b=5)
            for g in range(5):
                ps = small_tile([9, 200])
                nc.tensor.matmul(ps[:],
                                 wp[0:64, _O_ZI + 4 - g:_O_ZI + 13 - g],
                                 Qb5[:, :, g, :, :], start=True, stop=True)
                for bj in range(2):
                    nc.vector.tensor_add(
                        CI5[:, 2 * bj:2 * bj + 2, g, :, :],
                        ps[:].rearrange("p (w k b c) -> p w k b c",
                                        w=2, k=2, b=5)[:, bj, :, :, :],
                        mi5[:, 2 * bj:2 * bj + 2, g, :, :])

            CJ = const.tile([14, 1000], bf16, tag="CJ")
            CJ5 = CJ[:].rearrange("p (w a b c) -> p w a b c", w=4, a=5, b=5)
            mj5 = wp[0:14, _O_MJ:_O_MJ + 1000].rearrange(
                "p (w a b c) -> p w a b c", w=4, a=5, b=5)
            for bj in range(2):
                ps = big_tile([14, 500])
                for g in range(5):
                    nc.tensor.matmul(
                        ps[:, 100 * g:100 * (g + 1)],
                        wp[0:64, _O_ZJ + 9 - (5 * bj + g):
                           _O_ZJ + 23 - (5 * bj + g)],
                        Qb5[:, 2 * bj:2 * bj + 2, :, g, :],
                        start=True, stop=True)
                for bk in range(2):
                    nc.vector.tensor_add(
                        CJ5[:, 2 * bj + bk, :, :, :],
                        ps[:].rearrange("p (g k a c) -> p k a g c",
                                        g=5, k=2, a=5)[:, bk, :, :, :],
                        mj5[:, 2 * bj + bk, :, :, :])

            CK = const.tile([24, 1000], bf16, tag="CK")
            CK5 = CK[:].rearrange("p (w a b c) -> p w a b c", w=4, a=5, b=5)
            mk5 = wp[0:24, _O_MK:_O_MK + 1000].rearrange(
                "p (w a b c) -> p w a b c", w=4, a=5, b=5)
            for bk in range(2):
                ps = big_tile([24, 500])
                for g in range(10):
                    nc.tensor.matmul(
                        ps[:, 50 * g:50 * (g + 1)],
                        wp[0:64, _O_ZK + 19 - (10 * bk + g):
                           _O_ZK + 43 - (10 * bk + g)],
                        Qb5[:, bk:bk + 3:2, :, :, g], start=True, stop=True)
                for bj in range(2):
                    nc.vector.tensor_add(
                        CK5[:, 2 * bj + bk, :, :, :],
                        ps[:].rearrange("p (g w a b) -> p w a b g",
                                        g=10, w=2, a=5)[:, bj, :, :, :],
                        mk5[:, 2 * bj + bk, :, :, :])

            # ---- K projection ----
            KPT = const.tile([64, 3024], bf16, tag="KPT")
            for c in range(6):
                sl = slice(504 * c, 504 * (c + 1))
                ps = big_tile([64, 504])
                nc.tensor.matmul(ps[:], wp[0:65, _O_WK:_O_WK + 64],
                                 seT[:, sl], start=True, stop=True)
                drain(KPT[:, sl], ps[:])
            KPT4 = KPT[:].rearrange("p (i j k) -> p i j k", i=9, j=14)

            # ---- value table (s-major, (k,j,i) raster + static cols) ----
            for t in range(6):
                ps = big_tile([126, 256])
                for u in range(4):
                    c = 4 * t + u
                    nc.tensor.matmul(ps[:, 64 * u:64 * (u + 1)],
                                     seTk[:, 126 * c:126 * (c + 1)],
                                     wp[0:65, _O_WV:_O_WV + 64],
                                     start=True, stop=True)
                drain(vsm[:].rearrange("p (c e) -> p c e", c=24)
                      [:, 4 * t:4 * t + 4, 0:64], ps[:].rearrange(
                          "p (c e) -> p c e", c=4))
            vpsm = dpool.tile([3024, 128], bf16, tag="vpsm")
            nc.sync.dma_start(
                vpsm[:].rearrange("(c p) e -> p c e", c=24),
                vsm[:].rearrange("p (c e) -> p c e", c=24))
            # gather view: row (k*126 + j*9 + i), (i ch) merged contiguous
            vpj = vpsm[:].rearrange("(k j i) ch -> j k (i ch)", k=24, j=14)

            osb = const.tile([125, 8 * 64], f32, tag="osb")

            # ---- per-block stages ----
            def stage_A(blk):
                bj, bk = blk // 2, blk % 2
                pp = blk % 3
                jsl = slice(5 * bj, 5 * bj + 9)
                ksl = slice(10 * bk, 10 * bk + 14)
                kpd = kp[0:64, 1134 * pp:1134 * (pp + 1)].rearrange(
                    "p (i a c) -> p i a c", i=9, a=9)
                nc.vector.tensor_copy(kpd[:, 0:5, :, :],
                                      KPT4[:, 0:5, jsl, ksl])
                nc.scalar.copy(kpd[:, 5:9, :, :], KPT4[:, 5:9, jsl, ksl])
                vp = sVP.tile([126, 9 * 128], bf16, tag="vp")
                nc.sync.dma_start(vp[:], vpj[jsl, ksl, :])
                qa = sQA.tile([96, 250], bf16, tag="qa")
                bsl = slice(250 * blk, 250 * (blk + 1))
                nc.sync.dma_start(qa[0:64, :], Qb[:, bsl])
                nc.sync.dma_start(qa[64:73, :], CI[0:9, bsl])
                nc.sync.dma_start(qa[73:82, :], CJ[jsl, bsl])
                nc.sync.dma_start(qa[82:96, :], CK[ksl, bsl])
                return pp, vp, qa

            def stage_QKT(blk, st):
                pp, vp, qa = st
                Ss = []
                for t in range(5):
                    w = 500 if t < 4 else 250
                    pl = psL.tile([126, 500], f32, tag="L")
                    for u in range(2 if t < 4 else 1):
                        si = 2 * t + u
                        nc.tensor.matmul(
                            pl[:, 250 * u:250 * (u + 1)],
                            kp[0:96, 1134 * pp + 126 * si:
                               1134 * pp + 126 * (si + 1)],
                            qa[:], start=True, stop=True)
                    Sg = sS.tile([126, 500], bf16, tag="S")
                    nc.scalar.activation(Sg[:, 0:w], pl[:, 0:w], AF.Exp)
                    Ss.append(Sg)
                return Ss

            def stage_AVM(blk, st, Ss):
                pp, vp, qa = st
                # psv rows: 0:64 AV, 64 Z, 65:74 m2, 74:88 m1, 96:120 m3
                psv = psV.tile([128, 250], f32, tag="v")
                for si in range(9):
                    Ssl = Ss[si // 2][:, 250 * (si % 2):250 * (si % 2) + 250]
                    nc.tensor.matmul(psv[:], vp[:, 128 * si:128 * (si + 1)],
                                     Ssl, start=(si == 0), stop=False)
                msb = sW.tile([32, 250], bf16, tag="msb")
                nc.vector.tensor_copy(msb[:], psv[64:96, :])
                msbK = sW.tile([24, 250], f32, tag="msbK")
                nc.vector.tensor_copy(msbK[:], psv[96:120, :])
                Zsb = sW.tile([1, 250], f32, tag="Zsb")
                nc.vector.tensor_copy(Zsb[:], psv[64:65, :])
                return psv, msb, msbK, Zsb

            def stage_FIX(blk, avm):
                bj, bk = blk // 2, blk % 2
                psv, msb, msbK, Zsb = avm
                psvv = psv[:].rearrange("p (a b c) -> p a b c", a=5, b=5)
                msbv = msb[:].rearrange("p (a b c) -> p a b c", a=5, b=5)
                msbKv = msbK[:].rearrange("p (a b c) -> p a b c", a=5, b=5)
                for g in range(5):
                    nc.tensor.matmul(
                        psv[0:64, 50 * g:50 * (g + 1)],
                        wp[0:10, _O_RVI + 64 * g:_O_RVI + 64 * (g + 1)],
                        msb[0:10, 50 * g:50 * (g + 1)],
                        start=False, stop=False)
                for g in range(5):
                    u = 5 * bj + g
                    nc.tensor.matmul(
                        psvv[0:64, :, g, :],
                        wp[0:24, _O_RVJ + 64 * u:_O_RVJ + 64 * (u + 1)],
                        msbv[0:24, :, g, :],
                        start=False, stop=(g == 4))
                # k-axis: stride-10-inner operands need fp32 + own groups
                pk = psK.tile([64, 250], f32, tag="k")
                pkv = pk[:].rearrange("p (a b c) -> p a b c", a=5, b=5)
                for g in range(10):
                    u = 10 * bk + g
                    nc.tensor.matmul(
                        pkv[:, :, :, g],
                        fc[0:24, _F_RVK + 64 * u:_F_RVK + 64 * (u + 1)],
                        msbKv[:, :, :, g], start=True, stop=True)
                # per-block 1/Z: transpose Z halves on the PE, reciprocal
                psr = psS.tile([125, 2], f32, tag="x")
                nc.tensor.matmul(psr[:, 0:1], Zsb[:, 0:125], identf[:],
                                 start=True, stop=True)
                nc.tensor.matmul(psr[:, 1:2], Zsb[:, 125:250], identf[:],
                                 start=True, stop=True)
                rzt = sW.tile([125, 2], f32, tag="rzt")
                nc.vector.reciprocal(rzt[:], psr[:])
                ksb = sW.tile([64, 250], bf16, tag="ksb")
                nc.scalar.copy(ksb[:], pk[:])
                # avf rows 0:64 = AV + k-fixup; row 64 = Z (bias via Z*bo)
                avf = sW.tile([65, 250], bf16, tag="avf")
                nc.vector.tensor_add(avf[0:64, :], psv[0:64, :], ksb[:])
                nc.scalar.copy(avf[64:65, :], Zsb[:])
                return avf, rzt

            def stage_FIN(blk, fx):
                avf, rzt = fx
                for hh in range(2):
                    pso = psS.tile([125, 64], f32, tag="x")
                    nc.tensor.matmul(pso[:], avf[:, 125 * hh:125 * (hh + 1)],
                                     wp[0:65, _O_WO:_O_WO + 64],
                                     start=True, stop=True)
                    nc.scalar.mul(
                        osb[:, 64 * (2 * blk + hh):64 * (2 * blk + hh + 1)],
                        pso[:], rzt[:, hh:hh + 1])

            sts = {0: stage_A(0), 1: stage_A(1)}
            Ss = {0: stage_QKT(0, sts[0])}
            fx = {}
            for n in range(4):
                if n + 2 < 4:
                    sts[n + 2] = stage_A(n + 2)
                if n + 1 < 4:
                    Ss[n + 1] = stage_QKT(n + 1, sts[n + 1])
                avm = stage_AVM(n, sts[n], Ss[n])
                if n >= 1:
                    stage_FIN(n - 1, fx[n - 1])
                fx[n] = stage_FIX(n, avm)
            stage_FIN(3, fx[3])

            nc.sync.dma_start(out_p[:].rearrange("b q c -> q b c"), osb[:])

    nc.compile()
    _CACHE["nc"] = nc
    return nc


def _masks_for_core(bi, h):
    """Absolute-coordinate C masks MI [9,1000], MJ [14,1000], MK [24,1000]
    over the block-major query raster (blk, a, b, c) = (4, 5, 5, 10)."""
    q = np.arange(1000)
    blk, a, b, c = q // 250, (q // 50) % 5, (q // 10) % 5, q % 10
    bj, bk = blk // 2, blk % 2
    b_loc = 5 * bj + b
    c_loc = 10 * bk + c
    qi = 5 * bi + a
    qj = 10 * h + b_loc

    def vmask(qx, off):
        return (qx + off > 2) & (qx + off < 22)

    t9 = np.arange(9)[:, None]
    t14 = np.arange(14)[:, None]
    t24 = np.arange(24)[:, None]
    oi = t9 - a[None, :]
    oj = t14 - b_loc[None, :]
    ok = t24 - c_loc[None, :]
    wi = (oi >= 0) & (oi <= 4)
    wj = (oj >= 0) & (oj <= 4)
    wk = (ok >= 0) & (ok <= 4)
    # crossed validity (faithful to the reference): d2<->qj, d1<->qi
    MI = np.where(wi & vmask(qj[None, :], oi), 0.0, NEG)
    MJ = np.where(wj & vmask(qi[None, :], oj), 0.0, NEG)
    MK = np.where(wk & ((t24 > 2) & (t24 < 22)), 0.0, NEG)
    return MI, MJ, MK


def _bf16(x):
    import ml_dtypes
    return np.asarray(x, np.float32).astype(ml_dtypes.bfloat16)


def _pack_weights(inputs, bi, h):
    relpos = np.asarray(inputs["relpos_w"], np.float32)
    Wk = np.asarray(inputs["Wk"], np.float32)
    Wv = np.asarray(inputs["Wv"], np.float32)
    wpf = np.zeros((65, _WCOLS), np.float32)

    wpf[0:64, _O_WQ:_O_WQ + 64] = inputs["Wq"]
    wpf[64, _O_WQ:_O_WQ + 64] = inputs["bq"]
    wpf[0:64, _O_WK:_O_WK + 64] = Wk[96:160]
    wpf[64, _O_WK:_O_WK + 64] = inputs["bk"]
    wpf[0:64, _O_WV:_O_WV + 64] = Wv[96:160]
    wpf[64, _O_WV:_O_WV + 64] = inputs["bv"]
    wpf[0:64, _O_WO:_O_WO + 64] = inputs["Wo"]
    wpf[64, _O_WO:_O_WO + 64] = inputs["bo"]   # bias via Z*bo contract row

    G1 = relpos @ Wk[0:32]    # d1 (j-offsets)
    G2 = relpos @ Wk[32:64]   # d2 (i-offsets)
    G3 = relpos @ Wk[64:96]   # d3 (k-offsets)
    wpf[0:64, _O_ZI + 4:_O_ZI + 9] = G2.T
    wpf[0:64, _O_ZJ + 9:_O_ZJ + 14] = G1.T
    wpf[0:64, _O_ZK + 19:_O_ZK + 24] = G3.T

    RV1 = relpos @ Wv[0:32]
    RV2 = relpos @ Wv[32:64]
    # row 0 of each fixup tile is zero (multiplies the Z row of msb)
    for g in range(5):
        wpf[1 + g:1 + g + 5, _O_RVI + 64 * g:_O_RVI + 64 * (g + 1)] = RV2
    for bj in range(2):
        for g in range(5):
            u = 5 * bj + g
            wpf[10 + u:10 + u + 5,
                _O_RVJ + 64 * u:_O_RVJ + 64 * (u + 1)] = RV1

    MI, MJ, MK = _masks_for_core(bi, h)
    wpf[0:9, _O_MI:_O_MI + 1000] = MI
    wpf[0:14, _O_MJ:_O_MJ + 1000] = MJ
    wpf[0:24, _O_MK:_O_MK + 1000] = MK

    # kp indicator rows [32, 1134], tiled for the 3 ping-pong slots
    s = np.arange(9 * 9 * 14)
    s_i, s_j, s_k = s // 126, (s // 14) % 9, s % 14
    ind3 = np.zeros((32, 1134), np.float32)
    for t in range(9):
        ind3[t] = (s_i == t)
        ind3[9 + t] = (s_j == t)
    for t in range(14):
        ind3[18 + t] = (s_k == t)
    wpf[0:32, _O_IND:_WCOLS] = np.tile(ind3, (1, 3))
    return _bf16(wpf)


def _make_in_maps(inputs):
    se = np.asarray(inputs["spatial_embeddings"], np.float32)
    inputs = {k: np.asarray(v, np.float32) for k, v in inputs.items()}
    se_pad = np.pad(se, ((2, 2),) * 3 + ((0, 0),))
    relpos = inputs["relpos_w"]
    Wv = inputs["Wv"]

    # fconst: bo broadcast (unused cols kept) + RV3 k-fixup tiles
    fcf = np.zeros((125, _FCOLS), np.float32)
    fcf[:, 0:64] = np.broadcast_to(inputs["bo"], (125, 64))
    RV3 = relpos @ Wv[64:96]
    for bk in range(2):
        for g in range(10):
            u = 10 * bk + g
            fcf[u:u + 5, _F_RVK + 64 * u:_F_RVK + 64 * (u + 1)] = RV3

    # vind: static value-table columns [3024, 64] -> [126, 24*64]
    # cols: [1, d(t,li)x9, d(t,lj)x14, 0x8, d(t,lk)x24, 0x8]
    p = np.arange(3024)
    lk, lj, li = p // 126, (p // 9) % 14, p % 9
    vind = np.zeros((3024, 64), np.float32)
    vind[:, 0] = 1.0
    for t in range(9):
        vind[:, 1 + t] = (li == t)
    for t in range(14):
        vind[:, 10 + t] = (lj == t)
    for t in range(24):
        vind[:, 32 + t] = (lk == t)
    vind = vind.reshape(24, 126, 64).transpose(1, 0, 2).reshape(126, 1536)

    in_maps = []
    for core in range(8):
        bi, hh = core // 2, core % 2
        slab = se_pad[5 * bi:5 * bi + 9, 10 * hh:10 * hh + 14, :, :]
        seT65 = np.ones((65, 3024), np.float32)
        seT65[0:64] = slab.transpose(3, 0, 1, 2).reshape(64, 3024)
        seTk65 = np.ones((65, 3024), np.float32)
        seTk65[0:64] = slab.transpose(3, 2, 1, 0).reshape(64, 3024)
        m = dict(seT=_bf16(seT65), seTk=_bf16(seTk65),
                 wpack=_pack_weights(inputs, bi, hh),
                 vind=_bf16(vind), fc=fcf)
        in_maps.append(m)
    return in_maps


def _assemble(results):
    out = np.empty((20, 20, 20, 64), np.float32)
    for core in range(8):
        bi, hh = core // 2, core % 2
        chunks = np.asarray(results[core]["out"])  # [8,125,64]
        for blk in range(4):
            bj, bk = blk // 2, blk % 2
            bq = chunks[2 * blk:2 * blk + 2].reshape(5, 5, 10, 64)
            out[5 * bi:5 * bi + 5,
                10 * hh + 5 * bj:10 * hh + 5 * bj + 5,
                10 * bk:10 * bk + 10] = bq
    return out


def kernel(**inputs):
    import sys
    for pth in ("/opt/trn_rl_repo", "/root/.axon_site/_ro/trn_rl_repo"):
        if pth not in sys.path:
            sys.path.append(pth)
    from concourse.bass_utils import run_bass_kernel_spmd

    nc = _bass_mod()
    in_maps = _make_in_maps(inputs)
    res = run_bass_kernel_spmd(nc, in_maps, core_ids=list(range(8)))
    return _assemble(res.results)


# revision 37
# speedup vs baseline: 1.3603x; 1.3603x over previous
"""CubeAttention Trainium2 Bass kernel (8-core SPMD), v3.1.

Data-parallel over the query grid: the 20^3 grid splits into 8 slabs of
[5,10,20] (4 blocks along i x 2 halves along j). Each core processes 4
query blocks of [5i,5j,10k] (250 queries) against a 9x9x14 support
window, as 9 si-planes of [126 slots, 250 queries].

Key structure (vs the v2 291us kernel):
  - blocks of 250 queries: half the QKT/AV matmul count for the same
    streamed columns; 126-partition planes use the PE wider.
  - softmax marginals fused into the AV matmul: the s-major value table
    carries 64 static columns (ones + coordinate indicators at
    32-aligned rows), so one [126,128] x [126,250] matmul per plane
    yields AV, Z and all three axis marginals.
  - relpos-query C terms + masks computed once per core into
    block-major tables; per-block qa assembly is 4 plain 2D DMAs.
  - softmax normalisation deferred: Z rides the AV output (ones col),
    1/Z via two tiny per-block transposes, applied in the FIN drain;
    bias enters as a Z*bo contract row so nothing runs after FIN(3)
    except the output DMA.
  - no Pool-engine work (its copies/DMA-gen are SW-slow); PSUM spread
    over 4 pools (8 banks) so setup matmuls pipeline 8 drains deep.

Partition-base rule (engine APs must start at 0/32/64/96): marginal
rows sit at psum 64:96 and 96:120; the i/j fixup matmuls widen their
contract with zero lhsT rows to start at partition 0.
"""

import numpy as np

SCOPE, GN, D, CAP = 2, 20, 64, 32
NEG = np.float32(-1e9)

# wpack column layout (bf16, [65, _WCOLS])
_O_WQ, _O_WK, _O_WV, _O_WO = 0, 64, 128, 192
_O_ZI = 256            # [64, 13]  shifted-G2 (i-axis C), band cols 4:9
_O_ZJ = _O_ZI + 13     # [64, 23]  shifted-G1 (j-axis C), band cols 9:14
_O_ZK = _O_ZJ + 23     # [64, 43]  shifted-G3 (k-axis C), band cols 19:24
_O_RVI = _O_ZK + 43    # [10, 5*64]  RV2 fixup (g); row 0 zero (kills Z)
_O_RVJ = _O_RVI + 320  # [24, 10*64] RV1 fixup (5bj+g); rows 0:10 zero
_O_MI = _O_RVJ + 640   # [9, 1000]  i-axis C mask (block-major cols)
_O_MJ = _O_MI + 1000   # [14, 1000] j-axis C mask
_O_MK = _O_MJ + 1000   # [24, 1000] k-axis C mask
_O_IND = _O_MK + 1000  # [32, 3*1134] kp indicator rows (3 ping-pong)
_WCOLS = _O_IND + 3402
_W_SB = _O_IND         # columns of wpack staged in SBUF

# fconst layout (f32, [24, 1280]): RV3 k-fixup tiles (10bk+g)
_FCOLS = 1280

_CACHE = {}


def _bass_mod():
    if "nc" in _CACHE:
        return _CACHE["nc"]
    import sys
    for p in ("/opt/trn_rl_repo", "/root/.axon_site/_ro/trn_rl_repo"):
        if p not in sys.path:
            sys.path.append(p)
    import concourse.tile as tile
    from concourse import bacc, mybir

    f32 = mybir.dt.float32
    bf16 = mybir.dt.bfloat16
    AF = mybir.ActivationFunctionType

    nc = bacc.Bacc("TRN2", target_bir_lowering=False, debug=False)
    P = {}
    P["seT"] = nc.declare_dram_parameter("seT", [65, 3024], bf16, isOutput=False)
    P["seTk"] = nc.declare_dram_parameter("seTk", [65, 3024], bf16, isOutput=False)
    P["wpack"] = nc.declare_dram_parameter("wpack", [65, _WCOLS], bf16,
                                           isOutput=False)
    P["vind"] = nc.declare_dram_parameter("vind", [126, 24 * 64], bf16,
                                          isOutput=False)
    P["fc"] = nc.declare_dram_parameter("fc", [24, _FCOLS], f32, isOutput=False)
    out_p = nc.declare_dram_parameter("out", [8, 125, 64], f32, isOutput=True)

    with tile.TileContext(nc) as tc:
        with (
            tc.tile_pool(name="const", bufs=1) as const,
            tc.tile_pool(name="sS", bufs=8) as sS,
            tc.tile_pool(name="sVP", bufs=3) as sVP,
            tc.tile_pool(name="sQA", bufs=3) as sQA,
            tc.tile_pool(name="sW", bufs=2) as sW,
            tc.tile_pool(name="dram", bufs=1, space="DRAM") as dpool,
            tc.tile_pool(name="psS", bufs=2, space="PSUM") as psS,
            tc.tile_pool(name="psL", bufs=3, space="PSUM") as psL,
            tc.tile_pool(name="psV", bufs=2, space="PSUM") as psV,
            tc.tile_pool(name="psK", bufs=1, space="PSUM") as psK,
        ):
            # ---- input DMAs (spread across HWDGE queues, none on Pool) ----
            wp = const.tile([65, _O_MI], bf16, tag="wp")
            nc.scalar.dma_start(wp[:], P["wpack"][:, 0:_O_MI])
            seT = const.tile([65, 3024], bf16, tag="seT")
            nc.sync.dma_start(seT[:], P["seT"][:])
            seT4 = seT[:].rearrange("p (i j k) -> p i j k", i=9, j=14)
            seTk = const.tile([65, 3024], bf16, tag="seTk")
            nc.scalar.dma_start(seTk[:], P["seTk"][:])
            mp = const.tile([24, 3000], bf16, tag="mp")
            nc.sync.dma_start(mp[:], P["wpack"][0:24, _O_MI:_O_MI + 3000])
            fc = const.tile([24, _FCOLS], f32, tag="fc")
            nc.sync.dma_start(fc[:], P["fc"][:])
            kp = const.tile([96, 3 * 1134], bf16, tag="kp")
            nc.scalar.dma_start(kp[64:96, :], P["wpack"][0:32, _O_IND:_WCOLS])
            vsm = const.tile([126, 24 * 128], bf16, tag="vsm")
            nc.sync.dma_start(
                vsm[:].rearrange("p (c e) -> p c e", c=24)[:, :, 64:128],
                P["vind"][:].rearrange("p (c e) -> p c e", c=24))
            identf = const.tile([1, 1], f32, tag="identf")
            nc.vector.memset(identf[:], 1.0)

            dcnt = [0]

            def drain(dst, src):
                # alternate PSUM->SBUF copy engines
                dcnt[0] += 1
                if dcnt[0] % 2 == 0:
                    nc.scalar.copy(dst, src)
                else:
                    nc.vector.tensor_copy(dst, src)

            bigp = [0]

            def big_tile(shape):
                bigp[0] += 1
                pool = psS if bigp[0] % 2 == 0 else psL
                tag = "x" if pool is psS else "L"
                return pool.tile(shape, f32, tag=tag, name=f"bt{bigp[0]}")

            smlp = [0]

            def small_tile(shape):
                smlp[0] += 1
                pool = psV if smlp[0] % 3 else psK
                tag = "v" if pool is psV else "k"
                return pool.tile(shape, f32, tag=tag, name=f"st{smlp[0]}")

            # reserve slot sizes: first tile request fixes nothing, but
            # keep every request within [*,504] f32 (psS/psL) / [*,250] f32
            # (psV/psK).

            # ---- Q projection, block-major (blk, a, b, c) ----
            Qb = const.tile([64, 1000], bf16, tag="Qb")
            Qb5 = Qb[:].rearrange("p (w a b c) -> p w a b c", w=4, a=5, b=5)
            for a in range(5):
                ps = small_tile([64, 200])
                nc.tensor.matmul(ps[:], wp[0:65, _O_WQ:_O_WQ + 64],
                                 seT4[:, a + 2, 2:12, 2:22],
                                 start=True, stop=True)
                for bj in range(2):
                    drain(Qb5[:, 2 * bj:2 * bj + 2, a, :, :],
                          ps[:].rearrange("p (w b k c) -> p w k b c",
                                          w=2, b=5, k=2)[:, bj, :, :, :])

            # ---- relpos-query C tables + masks (block-major) ----
            CI = const.tile([9, 1000], bf16, tag="CI")
            CI5 = CI[:].rearrange("p (w a b c) -> p w a b c", w=4, a=5, b=5)
            mi5 = mp[0:9, 0:1000].rearrange(
                "p (w a b c) -> p w a b c", w=4, a=5, b=5)
            for g in range(5):
                ps = small_tile([9, 200])
                nc.tensor.matmul(ps[:],
                                 wp[0:64, _O_ZI + 4 - g:_O_ZI + 13 - g],
                                 Qb5[:, :, g, :, :], start=True, stop=True)
                for bj in range(2):
                    nc.vector.tensor_add(
                        CI5[:, 2 * bj:2 * bj + 2, g, :, :],
                        ps[:].rearrange("p (w k b c) -> p w k b c",
                                        w=2, k=2, b=5)[:, bj, :, :, :],
                        mi5[:, 2 * bj:2 * bj + 2, g, :, :])

            CJ = const.tile([14, 1000], bf16, tag="CJ")
            CJ5 = CJ[:].rearrange("p (w a b c) -> p w a b c", w=4, a=5, b=5)
            mj5 = mp[0:14, 1000:2000].rearrange(
                "p (w a b c) -> p w a b c", w=4, a=5, b=5)
            for bj in range(2):
                ps = big_tile([14, 500])
                for g in range(5):
                    nc.tensor.matmul(
                        ps[:, 100 * g:100 * (g + 1)],
                        wp[0:64, _O_ZJ + 9 - (5 * bj + g):
                           _O_ZJ + 23 - (5 * bj + g)],
                        Qb5[:, 2 * bj:2 * bj + 2, :, g, :],
                        start=True, stop=True)
                for bk in range(2):
                    nc.vector.tensor_add(
                        CJ5[:, 2 * bj + bk, :, :, :],
                        ps[:].rearrange("p (g k a c) -> p k a g c",
                                        g=5, k=2, a=5)[:, bk, :, :, :],
                        mj5[:, 2 * bj + bk, :, :, :])

            CK = const.tile([24, 1000], bf16, tag="CK")
            CK5 = CK[:].rearrange("p (w a b c) -> p w a b c", w=4, a=5, b=5)
            mk5 = mp[0:24, 2000:3000].rearrange(
                "p (w a b c) -> p w a b c", w=4, a=5, b=5)
            for bk in range(2):
                ps = big_tile([24, 500])
                for g in range(10):
                    nc.tensor.matmul(
                        ps[:, 50 * g:50 * (g + 1)],
                        wp[0:64, _O_ZK + 19 - (10 * bk + g):
                           _O_ZK + 43 - (10 * bk + g)],
                        Qb5[:, bk:bk + 3:2, :, :, g], start=True, stop=True)
                for bj in range(2):
                    nc.vector.tensor_add(
                        CK5[:, 2 * bj + bk, :, :, :],
                        ps[:].rearrange("p (g w a b) -> p w a b g",
                                        g=10, w=2, a=5)[:, bj, :, :, :],
                        mk5[:, 2 * bj + bk, :, :, :])

            # ---- K projection ----
            KPT = const.tile([64, 3024], bf16, tag="KPT")
            for c in range(6):
                sl = slice(504 * c, 504 * (c + 1))
                ps = big_tile([64, 504])
                nc.tensor.matmul(ps[:], wp[0:65, _O_WK:_O_WK + 64],
                                 seT[:, sl], start=True, stop=True)
                drain(KPT[:, sl], ps[:])
            KPT4 = KPT[:].rearrange("p (i j k) -> p i j k", i=9, j=14)

            # ---- value table (s-major, (k,j,i) raster + static cols) ----
            def vsm_chunk(t):
                ps = big_tile([126, 256])
                for u in range(4):
                    c = 4 * t + u
                    nc.tensor.matmul(ps[:, 64 * u:64 * (u + 1)],
                                     seTk[:, 126 * c:126 * (c + 1)],
                                     wp[0:65, _O_WV:_O_WV + 64],
                                     start=True, stop=True)
                drain(vsm[:].rearrange("p (c e) -> p c e", c=24)
                      [:, 4 * t:4 * t + 4, 0:64], ps[:].rearrange(
                          "p (c e) -> p c e", c=4))

            for t in range(3):
                vsm_chunk(t)
            vpsm = dpool.tile([3024, 128], bf16, tag="vpsm")
            # gather view: row (k*126 + j*9 + i), (i ch) merged contiguous
            vpj = vpsm[:].rearrange("(k j i) ch -> j k (i ch)", k=24, j=14)

            osb = const.tile([125, 8 * 64], f32, tag="osb")

            # ---- per-block stages ----
            def stage_vp(blk):
                bj, bk = blk // 2, blk % 2
                jsl = slice(5 * bj, 5 * bj + 9)
                ksl = slice(10 * bk, 10 * bk + 14)
                vp = sVP.tile([126, 9 * 128], bf16, tag="vp")
                nc.sync.dma_start(vp[:], vpj[jsl, ksl, :])
                return vp

            def stage_A(blk, vp=None):
                bj, bk = blk // 2, blk % 2
                pp = blk % 3
                jsl = slice(5 * bj, 5 * bj + 9)
                ksl = slice(10 * bk, 10 * bk + 14)
                kpd = kp[0:64, 1134 * pp:1134 * (pp + 1)].rearrange(
                    "p (i a c) -> p i a c", i=9, a=9)
                nc.vector.tensor_copy(kpd[:, 0:5, :, :],
                                      KPT4[:, 0:5, jsl, ksl])
                nc.scalar.copy(kpd[:, 5:9, :, :], KPT4[:, 5:9, jsl, ksl])
                if vp is None:
                    vp = stage_vp(blk)   # vp=False: caller gathers later
                qa = sQA.tile([96, 250], bf16, tag="qa")
                bsl = slice(250 * blk, 250 * (blk + 1))
                nc.vector.tensor_copy(qa[0:64, :], Qb[:, bsl])
                nc.vector.tensor_copy(qa[64:73, :], CI[0:9, bsl])
                nc.sync.dma_start(qa[73:82, :], CJ[jsl, bsl])
                nc.sync.dma_start(qa[82:96, :], CK[ksl, bsl])
                return pp, vp, qa

            def stage_QKT(blk, st):
                pp, vp, qa = st
                Ss = []
                for t in range(5):
                    w = 500 if t < 4 else 250
                    pl = psL.tile([126, 500], f32, tag="L")
                    for u in range(2 if t < 4 else 1):
                        si = 2 * t + u
                        nc.tensor.matmul(
                            pl[:, 250 * u:250 * (u + 1)],
                            kp[0:96, 1134 * pp + 126 * si:
                               1134 * pp + 126 * (si + 1)],
                            qa[:], start=True, stop=True)
                    Sg = sS.tile([126, 500], bf16, tag="S")
                    nc.scalar.activation(Sg[:, 0:w], pl[:, 0:w], AF.Exp)
                    Ss.append(Sg)
                return Ss

            def stage_AVM(blk, st, Ss):
                pp, vp, qa = st
                # psv rows: 0:64 AV, 64 Z, 65:74 m2, 74:88 m1, 96:120 m3
                psv = psV.tile([128, 250], f32, tag="v")
                for si in range(9):
                    Ssl = Ss[si // 2][:, 250 * (si % 2):250 * (si % 2) + 250]
                    nc.tensor.matmul(psv[:], vp[:, 128 * si:128 * (si + 1)],
                                     Ssl, start=(si == 0), stop=False)
                msb = sW.tile([32, 250], bf16, tag="msb")
                nc.vector.tensor_copy(msb[:], psv[64:96, :])
                msbK = sW.tile([24, 250], f32, tag="msbK")
                nc.vector.tensor_copy(msbK[:], psv[96:120, :])
                Zsb = sW.tile([1, 250], f32, tag="Zsb")
                nc.vector.tensor_copy(Zsb[:], psv[64:65, :])
                return psv, msb, msbK, Zsb

            def stage_FIX(blk, avm):
                bj, bk = blk // 2, blk % 2
                psv, msb, msbK, Zsb = avm
                psvv = psv[:].rearrange("p (a b c) -> p a b c", a=5, b=5)
                msbv = msb[:].rearrange("p (a b c) -> p a b c", a=5, b=5)
                msbKv = msbK[:].rearrange("p (a b c) -> p a b c", a=5, b=5)
                for g in range(5):
                    nc.tensor.matmul(
                        psv[0:64, 50 * g:50 * (g + 1)],
                        wp[0:10, _O_RVI + 64 * g:_O_RVI + 64 * (g + 1)],
                        msb[0:10, 50 * g:50 * (g + 1)],
                        start=False, stop=False)
                for g in range(5):
                    u = 5 * bj + g
                    nc.tensor.matmul(
                        psvv[0:64, :, g, :],
                        wp[0:24, _O_RVJ + 64 * u:_O_RVJ + 64 * (u + 1)],
                        msbv[0:24, :, g, :],
                        start=False, stop=(g == 4))
                # k-axis: stride-10-inner operands need fp32 + own groups
                pk = psK.tile([64, 250], f32, tag="k")
                pkv = pk[:].rearrange("p (a b c) -> p a b c", a=5, b=5)
                for g in range(10):
                    u = 10 * bk + g
                    nc.tensor.matmul(
                        pkv[:, :, :, g],
                        fc[0:24, 64 * u:64 * (u + 1)],
                        msbKv[:, :, :, g], start=True, stop=True)
                # per-block 1/Z: transpose Z halves on the PE, reciprocal
                psr = psS.tile([125, 2], f32, tag="x")
                nc.tensor.matmul(psr[:, 0:1], Zsb[:, 0:125], identf[:],
                                 start=True, stop=True)
                nc.tensor.matmul(psr[:, 1:2], Zsb[:, 125:250], identf[:],
                                 start=True, stop=True)
                rzt = sW.tile([125, 2], f32, tag="rzt")
                nc.vector.reciprocal(rzt[:], psr[:])
                ksb = sW.tile([64, 250], bf16, tag="ksb")
                nc.scalar.copy(ksb[:], pk[:])
                # avf rows 0:64 = AV + k-fixup; row 64 = Z (bias via Z*bo)
                avf = sW.tile([65, 250], bf16, tag="avf")
                nc.vector.tensor_add(avf[0:64, :], psv[0:64, :], ksb[:])
                nc.scalar.copy(avf[64:65, :], Zsb[:])
                return avf, rzt

            def stage_FIN(blk, fx):
                avf, rzt = fx
                for hh in range(2):
                    pso = psS.tile([125, 64], f32, tag="x")
                    nc.tensor.matmul(pso[:], avf[:, 125 * hh:125 * (hh + 1)],
                                     wp[0:65, _O_WO:_O_WO + 64],
                                     start=True, stop=True)
                    nc.scalar.mul(
                        osb[:, 64 * (2 * blk + hh):64 * (2 * blk + hh + 1)],
                        pso[:], rzt[:, hh:hh + 1])
                # stream this block's output chunk out
                nc.sync.dma_start(
                    out_p[2 * blk:2 * blk + 2, :, :].rearrange(
                        "b q c -> q b c"),
                    osb[:, 128 * blk:128 * (blk + 1)].rearrange(
                        "q (b c) -> q b c", b=2))

            sts = {0: stage_A(0, vp=False), 1: stage_A(1, vp=False)}
            Ss = {0: stage_QKT(0, sts[0])}
            # remaining value-table chunks overlap the first block's QKT
            for t in range(3, 6):
                vsm_chunk(t)
            nc.sync.dma_start(
                vpsm[:].rearrange("(c p) e -> p c e", c=24),
                vsm[:].rearrange("p (c e) -> p c e", c=24))
            sts[0] = (sts[0][0], stage_vp(0), sts[0][2])
            sts[1] = (sts[1][0], stage_vp(1), sts[1][2])
            fx = {}
            for n in range(4):
                if n + 2 < 4:
                    sts[n + 2] = stage_A(n + 2)
                if n + 1 < 4:
                    Ss[n + 1] = stage_QKT(n + 1, sts[n + 1])
                avm = stage_AVM(n, sts[n], Ss[n])
                if n >= 1:
                    stage_FIN(n - 1, fx[n - 1])
                fx[n] = stage_FIX(n, avm)
            stage_FIN(3, fx[3])

    nc.compile()
    _CACHE["nc"] = nc
    return nc


def _masks_for_core(bi, h):
    """Absolute-coordinate C masks MI [9,1000], MJ [14,1000], MK [24,1000]
    over the block-major query raster (blk, a, b, c) = (4, 5, 5, 10)."""
    q = np.arange(1000)
    blk, a, b, c = q // 250, (q // 50) % 5, (q // 10) % 5, q % 10
    bj, bk = blk // 2, blk % 2
    b_loc = 5 * bj + b
    c_loc = 10 * bk + c
    qi = 5 * bi + a
    qj = 10 * h + b_loc

    def vmask(qx, off):
        return (qx + off > 2) & (qx + off < 22)

    t9 = np.arange(9)[:, None]
    t14 = np.arange(14)[:, None]
    t24 = np.arange(24)[:, None]
    oi = t9 - a[None, :]
    oj = t14 - b_loc[None, :]
    ok = t24 - c_loc[None, :]
    wi = (oi >= 0) & (oi <= 4)
    wj = (oj >= 0) & (oj <= 4)
    wk = (ok >= 0) & (ok <= 4)
    # crossed validity (faithful to the reference): d2<->qj, d1<->qi
    MI = np.where(wi & vmask(qj[None, :], oi), 0.0, NEG)
    MJ = np.where(wj & vmask(qi[None, :], oj), 0.0, NEG)
    MK = np.where(wk & ((t24 > 2) & (t24 < 22)), 0.0, NEG)
    return MI, MJ, MK


def _bf16(x):
    import ml_dtypes
    return np.asarray(x, np.float32).astype(ml_dtypes.bfloat16)


def _pack_weights(inputs, bi, h):
    relpos = np.asarray(inputs["relpos_w"], np.float32)
    Wk = np.asarray(inputs["Wk"], np.float32)
    Wv = np.asarray(inputs["Wv"], np.float32)
    wpf = np.zeros((65, _WCOLS), np.float32)

    wpf[0:64, _O_WQ:_O_WQ + 64] = inputs["Wq"]
    wpf[64, _O_WQ:_O_WQ + 64] = inputs["bq"]
    wpf[0:64, _O_WK:_O_WK + 64] = Wk[96:160]
    wpf[64, _O_WK:_O_WK + 64] = inputs["bk"]
    wpf[0:64, _O_WV:_O_WV + 64] = Wv[96:160]
    wpf[64, _O_WV:_O_WV + 64] = inputs["bv"]
    wpf[0:64, _O_WO:_O_WO + 64] = inputs["Wo"]
    wpf[64, _O_WO:_O_WO + 64] = inputs["bo"]   # bias via Z*bo contract row

    G1 = relpos @ Wk[0:32]    # d1 (j-offsets)
    G2 = relpos @ Wk[32:64]   # d2 (i-offsets)
    G3 = relpos @ Wk[64:96]   # d3 (k-offsets)
    wpf[0:64, _O_ZI + 4:_O_ZI + 9] = G2.T
    wpf[0:64, _O_ZJ + 9:_O_ZJ + 14] = G1.T
    wpf[0:64, _O_ZK + 19:_O_ZK + 24] = G3.T

    RV1 = relpos @ Wv[0:32]
    RV2 = relpos @ Wv[32:64]
    # row 0 of each fixup tile is zero (multiplies the Z row of msb)
    for g in range(5):
        wpf[1 + g:1 + g + 5, _O_RVI + 64 * g:_O_RVI + 64 * (g + 1)] = RV2
    for bj in range(2):
        for g in range(5):
            u = 5 * bj + g
            wpf[10 + u:10 + u + 5,
                _O_RVJ + 64 * u:_O_RVJ + 64 * (u + 1)] = RV1

    MI, MJ, MK = _masks_for_core(bi, h)
    wpf[0:9, _O_MI:_O_MI + 1000] = MI
    wpf[0:14, _O_MJ:_O_MJ + 1000] = MJ
    wpf[0:24, _O_MK:_O_MK + 1000] = MK

    # kp indicator rows [32, 1134], tiled for the 3 ping-pong slots
    s = np.arange(9 * 9 * 14)
    s_i, s_j, s_k = s // 126, (s // 14) % 9, s % 14
    ind3 = np.zeros((32, 1134), np.float32)
    for t in range(9):
        ind3[t] = (s_i == t)
        ind3[9 + t] = (s_j == t)
    for t in range(14):
        ind3[18 + t] = (s_k == t)
    wpf[0:32, _O_IND:_WCOLS] = np.tile(ind3, (1, 3))
    return _bf16(wpf)


def _make_in_maps(inputs):
    se = np.asarray(inputs["spatial_embeddings"], np.float32)
    inputs = {k: np.asarray(v, np.float32) for k, v in inputs.items()}
    se_pad = np.pad(se, ((2, 2),) * 3 + ((0, 0),))
    relpos = inputs["relpos_w"]
    Wv = inputs["Wv"]

    # fconst: RV3 k-fixup tiles (f32)
    fcf = np.zeros((24, _FCOLS), np.float32)
    RV3 = relpos @ Wv[64:96]
    for bk in range(2):
        for g in range(10):
            u = 10 * bk + g
            fcf[u:u + 5, 64 * u:64 * (u + 1)] = RV3

    # vind: static value-table columns [3024, 64] -> [126, 24*64]
    # cols: [1, d(t,li)x9, d(t,lj)x14, 0x8, d(t,lk)x24, 0x8]
    p = np.arange(3024)
    lk, lj, li = p // 126, (p // 9) % 14, p % 9
    vind = np.zeros((3024, 64), np.float32)
    vind[:, 0] = 1.0
    for t in range(9):
        vind[:, 1 + t] = (li == t)
    for t in range(14):
        vind[:, 10 + t] = (lj == t)
    for t in range(24):
        vind[:, 32 + t] = (lk == t)
    vind = vind.reshape(24, 126, 64).transpose(1, 0, 2).reshape(126, 1536)

    in_maps = []
    for core in range(8):
        bi, hh = core // 2, core % 2
        slab = se_pad[5 * bi:5 * bi + 9, 10 * hh:10 * hh + 14, :, :]
        seT65 = np.ones((65, 3024), np.float32)
        seT65[0:64] = slab.transpose(3, 0, 1, 2).reshape(64, 3024)
        seTk65 = np.ones((65, 3024), np.float32)
        seTk65[0:64] = slab.transpose(3, 2, 1, 0).reshape(64, 3024)
        m = dict(seT=_bf16(seT65), seTk=_bf16(seTk65),
                 wpack=_pack_weights(inputs, bi, hh),
                 vind=_bf16(vind), fc=fcf)
        in_maps.append(m)
    return in_maps


def _assemble(results):
    out = np.empty((20, 20, 20, 64), np.float32)
    for core in range(8):
        bi, hh = core // 2, core % 2
        chunks = np.asarray(results[core]["out"])  # [8,125,64]
        for blk in range(4):
            bj, bk = blk // 2, blk % 2
            bq = chunks[2 * blk:2 * blk + 2].reshape(5, 5, 10, 64)
            out[5 * bi:5 * bi + 5,
                10 * hh + 5 * bj:10 * hh + 5 * bj + 5,
                10 * bk:10 * bk + 10] = bq
    return out


def kernel(**inputs):
    import sys
    for pth in ("/opt/trn_rl_repo", "/root/.axon_site/_ro/trn_rl_repo"):
        if pth not in sys.path:
            sys.path.append(pth)
    from concourse.bass_utils import run_bass_kernel_spmd

    nc = _bass_mod()
    in_maps = _make_in_maps(inputs)
    res = run_bass_kernel_spmd(nc, in_maps, core_ids=list(range(8)))
    return _assemble(res.results)


# revision 39
# speedup vs baseline: 1.3699x; 1.0070x over previous
"""CubeAttention Trainium2 Bass kernel (8-core SPMD), v3.1.

Data-parallel over the query grid: the 20^3 grid splits into 8 slabs of
[5,10,20] (4 blocks along i x 2 halves along j). Each core processes 4
query blocks of [5i,5j,10k] (250 queries) against a 9x9x14 support
window, as 9 si-planes of [126 slots, 250 queries].

Key structure (vs the v2 291us kernel):
  - blocks of 250 queries: half the QKT/AV matmul count for the same
    streamed columns; 126-partition planes use the PE wider.
  - softmax marginals fused into the AV matmul: the s-major value table
    carries 64 static columns (ones + coordinate indicators at
    32-aligned rows), so one [126,128] x [126,250] matmul per plane
    yields AV, Z and all three axis marginals.
  - relpos-query C terms + masks computed once per core into
    block-major tables; per-block qa assembly is 4 plain 2D DMAs.
  - softmax normalisation deferred: Z rides the AV output (ones col),
    1/Z via two tiny per-block transposes, applied in the FIN drain;
    bias enters as a Z*bo contract row so nothing runs after FIN(3)
    except the output DMA.
  - no Pool-engine work (its copies/DMA-gen are SW-slow); PSUM spread
    over 4 pools (8 banks) so setup matmuls pipeline 8 drains deep.

Partition-base rule (engine APs must start at 0/32/64/96): marginal
rows sit at psum 64:96 and 96:120; the i/j fixup matmuls widen their
contract with zero lhsT rows to start at partition 0.
"""

import numpy as np

SCOPE, GN, D, CAP = 2, 20, 64, 32
NEG = np.float32(-1e9)

# wpack column layout (bf16, [65, _WCOLS])
_O_WQ, _O_WK, _O_WV, _O_WO = 0, 64, 128, 192
_O_ZI = 256            # [64, 13]  shifted-G2 (i-axis C), band cols 4:9
_O_ZJ = _O_ZI + 13     # [64, 23]  shifted-G1 (j-axis C), band cols 9:14
_O_ZK = _O_ZJ + 23     # [64, 43]  shifted-G3 (k-axis C), band cols 19:24
_O_RVI = _O_ZK + 43    # [10, 5*64]  RV2 fixup (g); row 0 zero (kills Z)
_O_RVJ = _O_RVI + 320  # [24, 10*64] RV1 fixup (5bj+g); rows 0:10 zero
_O_RVK = _O_RVJ + 640  # [24, 20*64] RV3 fixup (10bk+g), (c,a,b)-major rhs
_O_MI = _O_RVK + 1280  # [9, 1000]  i-axis C mask (block-major cols)
_O_MJ = _O_MI + 1000   # [14, 1000] j-axis C mask
_O_MK = _O_MJ + 1000   # [24, 1000] k-axis C mask
_O_IND = _O_MK + 1000  # [32, 3*1134] kp indicator rows (3 ping-pong)
_WCOLS = _O_IND + 3402
_W_SB = _O_IND         # columns of wpack staged in SBUF

_CACHE = {}


def _bass_mod():
    if "nc" in _CACHE:
        return _CACHE["nc"]
    import sys
    for p in ("/opt/trn_rl_repo", "/root/.axon_site/_ro/trn_rl_repo"):
        if p not in sys.path:
            sys.path.append(p)
    import concourse.tile as tile
    from concourse import bacc, mybir

    f32 = mybir.dt.float32
    bf16 = mybir.dt.bfloat16
    AF = mybir.ActivationFunctionType

    nc = bacc.Bacc("TRN2", target_bir_lowering=False, debug=False)
    P = {}
    P["seT"] = nc.declare_dram_parameter("seT", [65, 3024], bf16, isOutput=False)
    P["seTk"] = nc.declare_dram_parameter("seTk", [65, 3024], bf16, isOutput=False)
    P["wpack"] = nc.declare_dram_parameter("wpack", [65, _WCOLS], bf16,
                                           isOutput=False)
    P["vind"] = nc.declare_dram_parameter("vind", [126, 24 * 64], bf16,
                                          isOutput=False)
    out_p = nc.declare_dram_parameter("out", [8, 125, 64], f32, isOutput=True)

    with tile.TileContext(nc) as tc:
        with (
            tc.tile_pool(name="const", bufs=1) as const,
            tc.tile_pool(name="sS", bufs=8) as sS,
            tc.tile_pool(name="sVP", bufs=3) as sVP,
            tc.tile_pool(name="sQA", bufs=3) as sQA,
            tc.tile_pool(name="sW", bufs=2) as sW,
            tc.tile_pool(name="dram", bufs=1, space="DRAM") as dpool,
            tc.tile_pool(name="psS", bufs=2, space="PSUM") as psS,
            tc.tile_pool(name="psL", bufs=3, space="PSUM") as psL,
            tc.tile_pool(name="psV", bufs=2, space="PSUM") as psV,
            tc.tile_pool(name="psK", bufs=1, space="PSUM") as psK,
        ):
            # ---- input DMAs (spread across HWDGE queues, none on Pool) ----
            wp = const.tile([65, _O_MI], bf16, tag="wp")
            # projection weights first so the first matmuls start early
            nc.scalar.dma_start(wp[:, 0:256], P["wpack"][:, 0:256])
            nc.scalar.dma_start(wp[:, 256:_O_MI], P["wpack"][:, 256:_O_MI])
            seT = const.tile([65, 3024], bf16, tag="seT")
            nc.sync.dma_start(seT[:], P["seT"][:])
            seT4 = seT[:].rearrange("p (i j k) -> p i j k", i=9, j=14)
            seTk = const.tile([65, 3024], bf16, tag="seTk")
            nc.scalar.dma_start(seTk[:], P["seTk"][:])
            mp = const.tile([24, 3000], bf16, tag="mp")
            nc.sync.dma_start(mp[:], P["wpack"][0:24, _O_MI:_O_MI + 3000])
            kp = const.tile([96, 3 * 1134], bf16, tag="kp")
            nc.scalar.dma_start(kp[64:96, :], P["wpack"][0:32, _O_IND:_WCOLS])
            vsm = const.tile([126, 24 * 128], bf16, tag="vsm")
            nc.sync.dma_start(
                vsm[:].rearrange("p (c e) -> p c e", c=24)[:, :, 64:128],
                P["vind"][:].rearrange("p (c e) -> p c e", c=24))
            identf = const.tile([1, 1], f32, tag="identf")
            nc.vector.memset(identf[:], 1.0)

            dcnt = [0]

            def drain(dst, src):
                # alternate PSUM->SBUF copy engines
                dcnt[0] += 1
                if dcnt[0] % 2 == 0:
                    nc.scalar.copy(dst, src)
                else:
                    nc.vector.tensor_copy(dst, src)

            bigp = [0]

            def big_tile(shape):
                bigp[0] += 1
                pool = psS if bigp[0] % 2 == 0 else psL
                tag = "x" if pool is psS else "L"
                return pool.tile(shape, f32, tag=tag, name=f"bt{bigp[0]}")

            smlp = [0]

            def small_tile(shape):
                smlp[0] += 1
                pool = psV if smlp[0] % 3 else psK
                tag = "v" if pool is psV else "k"
                return pool.tile(shape, f32, tag=tag, name=f"st{smlp[0]}")

            # reserve slot sizes: first tile request fixes nothing, but
            # keep every request within [*,504] f32 (psS/psL) / [*,250] f32
            # (psV/psK).

            # ---- Q projection, block-major (blk, a, b, c) ----
            Qb = const.tile([64, 1000], bf16, tag="Qb")
            Qb5 = Qb[:].rearrange("p (w a b c) -> p w a b c", w=4, a=5, b=5)
            for a in range(5):
                ps = small_tile([64, 200])
                nc.tensor.matmul(ps[:], wp[0:65, _O_WQ:_O_WQ + 64],
                                 seT4[:, a + 2, 2:12, 2:22],
                                 start=True, stop=True)
                for bj in range(2):
                    drain(Qb5[:, 2 * bj:2 * bj + 2, a, :, :],
                          ps[:].rearrange("p (w b k c) -> p w k b c",
                                          w=2, b=5, k=2)[:, bj, :, :, :])

            # ---- relpos-query C tables + masks (block-major) ----
            CI = const.tile([9, 1000], bf16, tag="CI")
            CI5 = CI[:].rearrange("p (w a b c) -> p w a b c", w=4, a=5, b=5)
            mi5 = mp[0:9, 0:1000].rearrange(
                "p (w a b c) -> p w a b c", w=4, a=5, b=5)
            for g in range(5):
                ps = small_tile([9, 200])
                nc.tensor.matmul(ps[:],
                                 wp[0:64, _O_ZI + 4 - g:_O_ZI + 13 - g],
                                 Qb5[:, :, g, :, :], start=True, stop=True)
                for bj in range(2):
                    nc.vector.tensor_add(
                        CI5[:, 2 * bj:2 * bj + 2, g, :, :],
                        ps[:].rearrange("p (w k b c) -> p w k b c",
                                        w=2, k=2, b=5)[:, bj, :, :, :],
                        mi5[:, 2 * bj:2 * bj + 2, g, :, :])

            CJ = const.tile([14, 1000], bf16, tag="CJ")
            CJ5 = CJ[:].rearrange("p (w a b c) -> p w a b c", w=4, a=5, b=5)
            mj5 = mp[0:14, 1000:2000].rearrange(
                "p (w a b c) -> p w a b c", w=4, a=5, b=5)
            for bj in range(2):
                ps = big_tile([14, 500])
                for g in range(5):
                    nc.tensor.matmul(
                        ps[:, 100 * g:100 * (g + 1)],
                        wp[0:64, _O_ZJ + 9 - (5 * bj + g):
                           _O_ZJ + 23 - (5 * bj + g)],
                        Qb5[:, 2 * bj:2 * bj + 2, :, g, :],
                        start=True, stop=True)
                for bk in range(2):
                    nc.vector.tensor_add(
                        CJ5[:, 2 * bj + bk, :, :, :],
                        ps[:].rearrange("p (g k a c) -> p k a g c",
                                        g=5, k=2, a=5)[:, bk, :, :, :],
                        mj5[:, 2 * bj + bk, :, :, :])

            CK = const.tile([24, 1000], bf16, tag="CK")
            CK5 = CK[:].rearrange("p (w a b c) -> p w a b c", w=4, a=5, b=5)
            mk5 = mp[0:24, 2000:3000].rearrange(
                "p (w a b c) -> p w a b c", w=4, a=5, b=5)
            for bk in range(2):
                ps = big_tile([24, 500])
                for g in range(10):
                    nc.tensor.matmul(
                        ps[:, 50 * g:50 * (g + 1)],
                        wp[0:64, _O_ZK + 19 - (10 * bk + g):
                           _O_ZK + 43 - (10 * bk + g)],
                        Qb5[:, bk:bk + 3:2, :, :, g], start=True, stop=True)
                for bj in range(2):
                    nc.vector.tensor_add(
                        CK5[:, 2 * bj + bk, :, :, :],
                        ps[:].rearrange("p (g w a b) -> p w a b g",
                                        g=10, w=2, a=5)[:, bj, :, :, :],
                        mk5[:, 2 * bj + bk, :, :, :])

            # ---- K projection (tile declared here, matmuls emitted
            # after the C section so block-0 qa DMAs overlap them) ----
            KPT = const.tile([64, 3024], bf16, tag="KPT")
            KPT4 = KPT[:].rearrange("p (i j k) -> p i j k", i=9, j=14)

            def kpt_chunk(c):
                sl = slice(504 * c, 504 * (c + 1))
                ps = big_tile([64, 504])
                nc.tensor.matmul(ps[:], wp[0:65, _O_WK:_O_WK + 64],
                                 seT[:, sl], start=True, stop=True)
                drain(KPT[:, sl], ps[:])

            # ---- value table (s-major, (k,j,i) raster + static cols) ----
            def vsm_chunk(t):
                ps = big_tile([126, 256])
                for u in range(4):
                    c = 4 * t + u
                    nc.tensor.matmul(ps[:, 64 * u:64 * (u + 1)],
                                     seTk[:, 126 * c:126 * (c + 1)],
                                     wp[0:65, _O_WV:_O_WV + 64],
                                     start=True, stop=True)
                drain(vsm[:].rearrange("p (c e) -> p c e", c=24)
                      [:, 4 * t:4 * t + 4, 0:64], ps[:].rearrange(
                          "p (c e) -> p c e", c=4))

            vpsm = dpool.tile([3024, 128], bf16, tag="vpsm")
            # gather view: row (k*126 + j*9 + i), (i ch) merged contiguous
            vpj = vpsm[:].rearrange("(k j i) ch -> j k (i ch)", k=24, j=14)

            osb = const.tile([125, 8 * 64], f32, tag="osb")

            # ---- per-block stages ----
            def stage_vp(blk):
                bj, bk = blk // 2, blk % 2
                jsl = slice(5 * bj, 5 * bj + 9)
                ksl = slice(10 * bk, 10 * bk + 14)
                vp = sVP.tile([126, 9 * 128], bf16, tag="vp")
                nc.sync.dma_start(vp[:], vpj[jsl, ksl, :])
                return vp

            def stage_kp(blk):
                bj, bk = blk // 2, blk % 2
                pp = blk % 3
                jsl = slice(5 * bj, 5 * bj + 9)
                ksl = slice(10 * bk, 10 * bk + 14)
                kpd = kp[0:64, 1134 * pp:1134 * (pp + 1)].rearrange(
                    "p (i a c) -> p i a c", i=9, a=9)
                nc.vector.tensor_copy(kpd[:, 0:5, :, :],
                                      KPT4[:, 0:5, jsl, ksl])
                nc.scalar.copy(kpd[:, 5:9, :, :], KPT4[:, 5:9, jsl, ksl])
                return pp

            def stage_qa(blk):
                bj, bk = blk // 2, blk % 2
                jsl = slice(5 * bj, 5 * bj + 9)
                ksl = slice(10 * bk, 10 * bk + 14)
                qa = sQA.tile([96, 250], bf16, tag="qa")
                bsl = slice(250 * blk, 250 * (blk + 1))
                nc.vector.tensor_copy(qa[0:64, :], Qb[:, bsl])
                nc.vector.tensor_copy(qa[64:73, :], CI[0:9, bsl])
                nc.sync.dma_start(qa[73:82, :], CJ[jsl, bsl])
                nc.sync.dma_start(qa[82:96, :], CK[ksl, bsl])
                return qa

            def stage_A(blk):
                qa = stage_qa(blk)
                pp = stage_kp(blk)
                vp = stage_vp(blk)
                return pp, vp, qa

            def stage_QKT(blk, st):
                pp, vp, qa = st
                Ss = []
                for t in range(5):
                    w = 500 if t < 4 else 250
                    pl = psL.tile([126, 500], f32, tag="L")
                    for u in range(2 if t < 4 else 1):
                        si = 2 * t + u
                        nc.tensor.matmul(
                            pl[:, 250 * u:250 * (u + 1)],
                            kp[0:96, 1134 * pp + 126 * si:
                               1134 * pp + 126 * (si + 1)],
                            qa[:], start=True, stop=True)
                    Sg = sS.tile([126, 500], bf16, tag="S")
                    nc.scalar.activation(Sg[:, 0:w], pl[:, 0:w], AF.Exp)
                    Ss.append(Sg)
                return Ss

            def stage_AVM(blk, st, Ss):
                pp, vp, qa = st
                # psv rows: 0:64 AV, 64 Z, 65:74 m2, 74:88 m1, 96:120 m3
                psv = psV.tile([128, 250], f32, tag="v")
                for si in range(9):
                    Ssl = Ss[si // 2][:, 250 * (si % 2):250 * (si % 2) + 250]
                    nc.tensor.matmul(psv[:], vp[:, 128 * si:128 * (si + 1)],
                                     Ssl, start=(si == 0), stop=False)
                msb = sW.tile([32, 250], bf16, tag="msb")
                nc.vector.tensor_copy(msb[:], psv[64:96, :])
                msbK = sW.tile([24, 250], bf16, tag="msbK")
                nc.vector.tensor_copy(msbK[:], psv[96:120, :].rearrange(
                    "p (a b c) -> p c a b", a=5, b=5))
                Zsb = sW.tile([1, 250], f32, tag="Zsb")
                nc.vector.tensor_copy(Zsb[:], psv[64:65, :])
                return psv, msb, msbK, Zsb

            def stage_FIX(blk, avm):
                bj, bk = blk // 2, blk % 2
                psv, msb, msbK, Zsb = avm
                psvv = psv[:].rearrange("p (a b c) -> p a b c", a=5, b=5)
                msbv = msb[:].rearrange("p (a b c) -> p a b c", a=5, b=5)
                for g in range(5):
                    nc.tensor.matmul(
                        psv[0:64, 50 * g:50 * (g + 1)],
                        wp[0:10, _O_RVI + 64 * g:_O_RVI + 64 * (g + 1)],
                        msb[0:10, 50 * g:50 * (g + 1)],
                        start=False, stop=False)
                for g in range(5):
                    u = 5 * bj + g
                    nc.tensor.matmul(
                        psvv[0:64, :, g, :],
                        wp[0:24, _O_RVJ + 64 * u:_O_RVJ + 64 * (u + 1)],
                        msbv[0:24, :, g, :],
                        start=False, stop=(g == 4))
                # k-axis fixup in its own (c,a,b)-major psum tile so every
                # operand is contiguous-inner (bf16-safe)
                pk = psK.tile([64, 250], f32, tag="k")
                for g in range(10):
                    u = 10 * bk + g
                    nc.tensor.matmul(
                        pk[:, 25 * g:25 * (g + 1)],
                        wp[0:24, _O_RVK + 64 * u:_O_RVK + 64 * (u + 1)],
                        msbK[0:24, 25 * g:25 * (g + 1)],
                        start=True, stop=True)
                # per-block 1/Z: transpose Z halves on the PE, reciprocal
                psr = psS.tile([125, 2], f32, tag="x")
                nc.tensor.matmul(psr[:, 0:1], Zsb[:, 0:125], identf[:],
                                 start=True, stop=True)
                nc.tensor.matmul(psr[:, 1:2], Zsb[:, 125:250], identf[:],
                                 start=True, stop=True)
                rzt = sW.tile([125, 2], f32, tag="rzt")
                nc.vector.reciprocal(rzt[:], psr[:])
                ksb = sW.tile([64, 250], bf16, tag="ksb")
                nc.scalar.copy(ksb[:], pk[:].rearrange(
                    "p (c a b) -> p a b c", c=10, a=5))
                # avf rows 0:64 = AV + k-fixup; row 64 = Z (bias via Z*bo)
                avf = sW.tile([65, 250], bf16, tag="avf")
                nc.vector.tensor_add(avf[0:64, :], psv[0:64, :], ksb[:])
                nc.scalar.copy(avf[64:65, :], Zsb[:])
                return avf, rzt

            def stage_FIN(blk, fx):
                avf, rzt = fx
                for hh in range(2):
                    pso = psS.tile([125, 64], f32, tag="x")
                    nc.tensor.matmul(pso[:], avf[:, 125 * hh:125 * (hh + 1)],
                                     wp[0:65, _O_WO:_O_WO + 64],
                                     start=True, stop=True)
                    nc.scalar.mul(
                        osb[:, 64 * (2 * blk + hh):64 * (2 * blk + hh + 1)],
                        pso[:], rzt[:, hh:hh + 1])
                # stream this block's output chunk out
                nc.sync.dma_start(
                    out_p[2 * blk:2 * blk + 2, :, :].rearrange(
                        "b q c -> q b c"),
                    osb[:, 128 * blk:128 * (blk + 1)].rearrange(
                        "q (b c) -> q b c", b=2))

            # ---- emission: qa(0/1) overlap KPT; QKT(0) overlaps vsm ----
            qa0 = stage_qa(0)
            qa1 = stage_qa(1)
            for c in range(6):
                kpt_chunk(c)
            pp0 = stage_kp(0)
            pp1 = stage_kp(1)
            sts = {0: (pp0, None, qa0), 1: (pp1, None, qa1)}
            Ss = {0: stage_QKT(0, sts[0])}
            for t in range(6):
                vsm_chunk(t)
            nc.sync.dma_start(
                vpsm[:].rearrange("(c p) e -> p c e", c=24),
                vsm[:].rearrange("p (c e) -> p c e", c=24))
            sts[0] = (pp0, stage_vp(0), qa0)
            sts[1] = (pp1, stage_vp(1), qa1)
            fx = {}
            for n in range(4):
                if n + 2 < 4:
                    sts[n + 2] = stage_A(n + 2)
                if n + 1 < 4:
                    Ss[n + 1] = stage_QKT(n + 1, sts[n + 1])
                avm = stage_AVM(n, sts[n], Ss[n])
                if n >= 1:
                    stage_FIN(n - 1, fx[n - 1])
                fx[n] = stage_FIX(n, avm)
            stage_FIN(3, fx[3])

    nc.compile()
    _CACHE["nc"] = nc
    return nc


def _masks_for_core(bi, h):
    """Absolute-coordinate C masks MI [9,1000], MJ [14,1000], MK [24,1000]
    over the block-major query raster (blk, a, b, c) = (4, 5, 5, 10)."""
    q = np.arange(1000)
    blk, a, b, c = q // 250, (q // 50) % 5, (q // 10) % 5, q % 10
    bj, bk = blk // 2, blk % 2
    b_loc = 5 * bj + b
    c_loc = 10 * bk + c
    qi = 5 * bi + a
    qj = 10 * h + b_loc

    def vmask(qx, off):
        return (qx + off > 2) & (qx + off < 22)

    t9 = np.arange(9)[:, None]
    t14 = np.arange(14)[:, None]
    t24 = np.arange(24)[:, None]
    oi = t9 - a[None, :]
    oj = t14 - b_loc[None, :]
    ok = t24 - c_loc[None, :]
    wi = (oi >= 0) & (oi <= 4)
    wj = (oj >= 0) & (oj <= 4)
    wk = (ok >= 0) & (ok <= 4)
    # crossed validity (faithful to the reference): d2<->qj, d1<->qi
    MI = np.where(wi & vmask(qj[None, :], oi), 0.0, NEG)
    MJ = np.where(wj & vmask(qi[None, :], oj), 0.0, NEG)
    MK = np.where(wk & ((t24 > 2) & (t24 < 22)), 0.0, NEG)
    return MI, MJ, MK


def _bf16(x):
    import ml_dtypes
    return np.asarray(x, np.float32).astype(ml_dtypes.bfloat16)


def _pack_weights(inputs, bi, h):
    relpos = np.asarray(inputs["relpos_w"], np.float32)
    Wk = np.asarray(inputs["Wk"], np.float32)
    Wv = np.asarray(inputs["Wv"], np.float32)
    wpf = np.zeros((65, _WCOLS), np.float32)

    wpf[0:64, _O_WQ:_O_WQ + 64] = inputs["Wq"]
    wpf[64, _O_WQ:_O_WQ + 64] = inputs["bq"]
    wpf[0:64, _O_WK:_O_WK + 64] = Wk[96:160]
    wpf[64, _O_WK:_O_WK + 64] = inputs["bk"]
    wpf[0:64, _O_WV:_O_WV + 64] = Wv[96:160]
    wpf[64, _O_WV:_O_WV + 64] = inputs["bv"]
    wpf[0:64, _O_WO:_O_WO + 64] = inputs["Wo"]
    wpf[64, _O_WO:_O_WO + 64] = inputs["bo"]   # bias via Z*bo contract row

    G1 = relpos @ Wk[0:32]    # d1 (j-offsets)
    G2 = relpos @ Wk[32:64]   # d2 (i-offsets)
    G3 = relpos @ Wk[64:96]   # d3 (k-offsets)
    wpf[0:64, _O_ZI + 4:_O_ZI + 9] = G2.T
    wpf[0:64, _O_ZJ + 9:_O_ZJ + 14] = G1.T
    wpf[0:64, _O_ZK + 19:_O_ZK + 24] = G3.T

    RV1 = relpos @ Wv[0:32]
    RV2 = relpos @ Wv[32:64]
    # row 0 of each fixup tile is zero (multiplies the Z row of msb)
    for g in range(5):
        wpf[1 + g:1 + g + 5, _O_RVI + 64 * g:_O_RVI + 64 * (g + 1)] = RV2
    for bj in range(2):
        for g in range(5):
            u = 5 * bj + g
            wpf[10 + u:10 + u + 5,
                _O_RVJ + 64 * u:_O_RVJ + 64 * (u + 1)] = RV1

    RV3 = relpos @ Wv[64:96]
    for bk in range(2):
        for g in range(10):
            u = 10 * bk + g
            wpf[10 * bk + g:10 * bk + g + 5,
                _O_RVK + 64 * u:_O_RVK + 64 * (u + 1)] = RV3

    MI, MJ, MK = _masks_for_core(bi, h)
    wpf[0:9, _O_MI:_O_MI + 1000] = MI
    wpf[0:14, _O_MJ:_O_MJ + 1000] = MJ
    wpf[0:24, _O_MK:_O_MK + 1000] = MK

    # kp indicator rows [32, 1134], tiled for the 3 ping-pong slots
    s = np.arange(9 * 9 * 14)
    s_i, s_j, s_k = s // 126, (s // 14) % 9, s % 14
    ind3 = np.zeros((32, 1134), np.float32)
    for t in range(9):
        ind3[t] = (s_i == t)
        ind3[9 + t] = (s_j == t)
    for t in range(14):
        ind3[18 + t] = (s_k == t)
    wpf[0:32, _O_IND:_WCOLS] = np.tile(ind3, (1, 3))
    return _bf16(wpf)


def _make_in_maps(inputs):
    se = np.asarray(inputs["spatial_embeddings"], np.float32)
    inputs = {k: np.asarray(v, np.float32) for k, v in inputs.items()}
    se_pad = np.pad(se, ((2, 2),) * 3 + ((0, 0),))

    # vind: static value-table columns [3024, 64] -> [126, 24*64]
    # cols: [1, d(t,li)x9, d(t,lj)x14, 0x8, d(t,lk)x24, 0x8]
    p = np.arange(3024)
    lk, lj, li = p // 126, (p // 9) % 14, p % 9
    vind = np.zeros((3024, 64), np.float32)
    vind[:, 0] = 1.0
    for t in range(9):
        vind[:, 1 + t] = (li == t)
    for t in range(14):
        vind[:, 10 + t] = (lj == t)
    for t in range(24):
        vind[:, 32 + t] = (lk == t)
    vind = vind.reshape(24, 126, 64).transpose(1, 0, 2).reshape(126, 1536)

    in_maps = []
    for core in range(8):
        bi, hh = core // 2, core % 2
        slab = se_pad[5 * bi:5 * bi + 9, 10 * hh:10 * hh + 14, :, :]
        seT65 = np.ones((65, 3024), np.float32)
        seT65[0:64] = slab.transpose(3, 0, 1, 2).reshape(64, 3024)
        seTk65 = np.ones((65, 3024), np.float32)
        seTk65[0:64] = slab.transpose(3, 2, 1, 0).reshape(64, 3024)
        m = dict(seT=_bf16(seT65), seTk=_bf16(seTk65),
                 wpack=_pack_weights(inputs, bi, hh),
                 vind=_bf16(vind))
        in_maps.append(m)
    return in_maps


def _assemble(results):
    out = np.empty((20, 20, 20, 64), np.float32)
    for core in range(8):
        bi, hh = core // 2, core % 2
        chunks = np.asarray(results[core]["out"])  # [8,125,64]
        for blk in range(4):
            bj, bk = blk // 2, blk % 2
            bq = chunks[2 * blk:2 * blk + 2].reshape(5, 5, 10, 64)
            out[5 * bi:5 * bi + 5,
                10 * hh + 5 * bj:10 * hh + 5 * bj + 5,
                10 * bk:10 * bk + 10] = bq
    return out


def kernel(**inputs):
    import sys
    for pth in ("/opt/trn_rl_repo", "/root/.axon_site/_ro/trn_rl_repo"):
        if pth not in sys.path:
            sys.path.append(pth)
    from concourse.bass_utils import run_bass_kernel_spmd

    nc = _bass_mod()
    in_maps = _make_in_maps(inputs)
    res = run_bass_kernel_spmd(nc, in_maps, core_ids=list(range(8)))
    return _assemble(res.results)
